# revision 5
# baseline (speedup 1.0000x reference)
"""Trainium2 Bass kernel v2 for multi-head attention (nn_Attention_39573828665669).

Same math as the baseline kernel (fp16 matmuls, unnormalized softmax via
E = exp(pb)*mask/256 and a ones-column in V for the denominator), restructured
for engine balance:

  - V is projected as V^T with 512-wide matmuls (like Q/K) and transposed to
    [t, m] layout on the PE (32 cheap [128,128] transposes) instead of the
    256-wide rhs=weights pass, which underfed the PE sequencer.
  - DMA triggers are batched 2 chunks per descriptor (8 KB/partition) and
    spread across engine queues: weights+E on Pool, hq/hkv on SP, out on ACT.
  - PSUM evacuations alternate ACT/DVE so neither engine serializes a stage.
  - Per-head softmax normalization computes 1/denom on a [2, TQB] tile
    (one Ln + one Exp on ACT instead of four), denom rows gathered by DVE.
"""

import os
from contextlib import ExitStack

import numpy as np

import concourse.bass as bass
import concourse.tile as tile
from concourse import bacc, mybir
from concourse.bass_utils import run_bass_kernel_spmd
from concourse.masks import make_identity

F32 = mybir.dt.float32
FP16 = mybir.dt.float16

ts = bass.ts

N_CORES = 8
H = 32
DH = 64
HPC = H // N_CORES          # heads per core = 4
M = HPC * DH                # per-core head dims = 256
E_SCALE = 1.0 / 256.0


def build_attention_nc(T: int, DM: int, reps: int = 1, with_bias: bool = False):
    assert HPC == 4 and DH == 64
    NJ = M // 128            # m chunks (2)
    NDC = DM // 128          # contraction chunks for projections
    NTK = T // 128           # key-position chunks
    NTQ = T // 512           # 512-wide query chunks
    TQB = min(1024, T)
    NTQB = T // TQB

    nc = bacc.Bacc("TRN2", target_bir_lowering=False, debug=False)

    d_hqT = nc.declare_dram_parameter("hqT", [DM, T], FP16, isOutput=False)
    d_hkvT = nc.declare_dram_parameter("hkvT", [DM, T], FP16, isOutput=False)
    d_wq = nc.declare_dram_parameter("wqT", [128, NDC * M], FP16, isOutput=False)
    d_wk = nc.declare_dram_parameter("wkT", [128, NDC * M], FP16, isOutput=False)
    d_wv = nc.declare_dram_parameter("wvT", [128, NDC * M], FP16, isOutput=False)
    d_wo = nc.declare_dram_parameter("woT", [128, NJ * DM], FP16, isOutput=False)
    d_bq = nc.declare_dram_parameter("bq", [M], F32, isOutput=False)
    d_bk = nc.declare_dram_parameter("bk", [M], F32, isOutput=False)
    d_E = nc.declare_dram_parameter("E", [HPC, T, T], FP16, isOutput=False)
    d_out = nc.declare_dram_parameter("out", [T, DM], FP16, isOutput=True)

    Exp = mybir.ActivationFunctionType.Exp
    Ln = mybir.ActivationFunctionType.Ln
    Copy = mybir.ActivationFunctionType.Copy
    Identity = mybir.ActivationFunctionType.Identity

    with tile.TileContext(nc) as tc, ExitStack() as ctx:
        wpool = ctx.enter_context(tc.tile_pool(name="weights", bufs=1))
        spool = ctx.enter_context(tc.tile_pool(name="state", bufs=1))

        wq_sb = wpool.tile([128, NDC, M], FP16, tag="wq")
        wk_sb = wpool.tile([128, NDC, M], FP16, tag="wk")
        wv_sb = wpool.tile([128, NDC, M], FP16, tag="wv")
        wo_sb = wpool.tile([128, NJ, DM], FP16, tag="wo")
        bq_sb = wpool.tile([128, NJ], F32, tag="bq")
        bk_sb = wpool.tile([128, NJ], F32, tag="bk")
        id_sb = wpool.tile([128, 128], FP16, tag="idn")
        # weights on the Pool queue so they overlap the first hq tiles (SP)
        nc.gpsimd.dma_start(
            wk_sb[:, 0:1, :].rearrange("p a b -> p (a b)"), d_wk.ap()[:, 0:M]
        )
        nc.gpsimd.dma_start(
            wv_sb[:, 0:1, :].rearrange("p a b -> p (a b)"), d_wv.ap()[:, 0:M]
        )
        nc.gpsimd.dma_start(
            wk_sb[:, 1:, :].rearrange("p a b -> p (a b)"), d_wk.ap()[:, M:]
        )
        nc.gpsimd.dma_start(
            wv_sb[:, 1:, :].rearrange("p a b -> p (a b)"), d_wv.ap()[:, M:]
        )
        nc.gpsimd.dma_start(wq_sb[:].rearrange("p a b -> p (a b)"), d_wq.ap())
        nc.gpsimd.dma_start(wo_sb[:].rearrange("p a b -> p (a b)"), d_wo.ap())
        nc.sync.dma_start(bq_sb[:], d_bq.ap().rearrange("(j p) -> p j", p=128))
        nc.sync.dma_start(bk_sb[:], d_bk.ap().rearrange("(j p) -> p j", p=128))
        make_identity(nc, id_sb[:])

        qt_sb = spool.tile([128, NJ, T], FP16, tag="qt")     # Q.T / sqrt(DH)
        kt_sb = spool.tile([128, NJ, T], FP16, tag="kt")     # K.T
        vt_sb = spool.tile([128, NJ, T], FP16, tag="vt")     # V.T (pre-transpose)
        VW = 128  # per-head AV lhsT width: [1 ones | 63 pad | 64 dh]
        vo_sb = spool.tile([128, NTK, HPC * VW], FP16, tag="vo")
        ot_sb = spool.tile([128, NJ, T], FP16, tag="ot")     # normalized O.T

        nc.vector.memset(vo_sb[:], 0.0)
        ones_view = vo_sb[:].rearrange("p n (h x) -> p n h x", x=VW)[:, :, :, 0:1]
        nc.vector.memset(ones_view, 1.0)

        def _evac(dst, src, b_sb, j, idx):
            if with_bias and b_sb is not None:
                nc.scalar.activation(dst, src, Identity, bias=b_sb[:, j : j + 1])
            elif idx % 2 == 0:
                nc.scalar.activation(dst, src, Copy)
            else:
                nc.vector.tensor_copy(dst, src)

        def _proj_pass(d_src, w_sb, b_sb, dst_sb, hin):
            # d-chunk-outer projection: full-T psum (NJ * NTQ = 8 banks),
            # one [128, 2, T] rhs DMA per dc pair (8 KB per partition line).
            with tc.tile_pool(name="pqk", bufs=8, space="PSUM") as pqk:
                pp = [
                    pqk.tile([128, 512], F32, tag="pp", name=f"pp{i}")
                    for i in range(NJ * NTQ)
                ]
                for dcp in range(NDC // 2):
                    ht = hin.tile([128, 2, T], FP16, tag="h")
                    nc.sync.dma_start(
                        ht[:],
                        d_src.ap()[ts(dcp, 256), :].rearrange(
                            "(a p) t -> p a t", a=2
                        ),
                    )
                    for a in range(2):
                        dc = dcp * 2 + a
                        for j in range(NJ):
                            for q in range(NTQ):
                                nc.tensor.matmul(
                                    pp[j * NTQ + q][:],
                                    lhsT=w_sb[:, dc, ts(j, 128)],
                                    rhs=ht[:, a, ts(q, 512)],
                                    start=(dc == 0),
                                    stop=(dc == NDC - 1),
                                )
                for j in range(NJ):
                    for q in range(NTQ):
                        _evac(
                            dst_sb[:, j, ts(q, 512)],
                            pp[j * NTQ + q][:],
                            b_sb,
                            j,
                            j * NTQ + q,
                        )

        def _proj_kv_fused():
            # K and V^T share one streaming pass over hkvT: half-T psum per
            # projection (4 banks K + 4 banks V^T), two column-halves.
            TH = T // 2
            for half in range(2):
                with (
                    tc.tile_pool(name="pkv", bufs=4, space="PSUM") as pkv,
                    tc.tile_pool(name="hin2", bufs=3) as hin2,
                ):
                    pk = [
                        pkv.tile([128, 512], F32, tag="pk", name=f"pk{half}_{i}")
                        for i in range(NJ * 2)
                    ]
                    pv = [
                        pkv.tile([128, 512], F32, tag="pv", name=f"pv{half}_{i}")
                        for i in range(NJ * 2)
                    ]
                    for dcp in range(NDC // 2):
                        ht = hin2.tile([128, 2, TH], FP16, tag="h2")
                        nc.sync.dma_start(
                            ht[:],
                            d_hkvT.ap()[
                                ts(dcp, 256), half * TH : (half + 1) * TH
                            ].rearrange("(a p) t -> p a t", a=2),
                        )
                        for a in range(2):
                            dc = dcp * 2 + a
                            for j in range(NJ):
                                for q in range(2):
                                    nc.tensor.matmul(
                                        pk[j * 2 + q][:],
                                        lhsT=wk_sb[:, dc, ts(j, 128)],
                                        rhs=ht[:, a, ts(q, 512)],
                                        start=(dc == 0),
                                        stop=(dc == NDC - 1),
                                    )
                                    nc.tensor.matmul(
                                        pv[j * 2 + q][:],
                                        lhsT=wv_sb[:, dc, ts(j, 128)],
                                        rhs=ht[:, a, ts(q, 512)],
                                        start=(dc == 0),
                                        stop=(dc == NDC - 1),
                                    )
                    for j in range(NJ):
                        for q in range(2):
                            col = half * TH + q * 512
                            _evac(
                                kt_sb[:, j, col : col + 512],
                                pk[j * 2 + q][:],
                                bk_sb,
                                j,
                                j * 2 + q,
                            )
                            _evac(
                                vt_sb[:, j, col : col + 512],
                                pv[j * 2 + q][:],
                                None,
                                j,
                                j * 2 + q + 1,
                            )

        def _transpose_v():
            # vt [m, t] -> vo [t, m] in [128,128] blocks on the PE.
            with tc.tile_pool(name="ptr", bufs=4, space="PSUM") as ptr:
                for tkc in range(NTK):
                    for j in range(NJ):
                        pt = ptr.tile([128, 128], FP16, tag="pt")
                        nc.tensor.transpose(
                            pt[:], vt_sb[:, j, ts(tkc, 128)], id_sb[:]
                        )
                        dst = vo_sb[:, tkc, :].rearrange(
                            "p (h x) -> p h x", x=VW
                        )[:, 2 * j : 2 * j + 2, 64 : 64 + DH]
                        nc.vector.tensor_copy(
                            dst, pt[:].rearrange("p (h d) -> p h d", d=DH)
                        )

        def _stage23():
            with (
                tc.tile_pool(name="sps", bufs=2, space="PSUM") as sps,
                tc.tile_pool(name="ops", bufs=2, space="PSUM") as ops,
                tc.tile_pool(name="epool", bufs=4) as epool,
                tc.tile_pool(name="xpool", bufs=4) as xpool,
                tc.tile_pool(name="npool", bufs=2) as npool,
            ):

                def _avs(h, otiles, tkc, pps):
                    for tqb in range(NTQB):
                        for q2 in range(TQB // 512):
                            nc.tensor.matmul(
                                otiles[tqb][:, ts(q2, 512)],
                                lhsT=vo_sb[:, tkc, h * VW : (h + 1) * VW],
                                rhs=pps[tqb][:, ts(q2, 512)],
                                start=(tkc == 0),
                                stop=(tkc == NTK - 1),
                            )

                for h in range(HPC):
                    j, hp = h // 2, 64 * (h % 2)
                    otiles = [
                        ops.tile([128, TQB], F32, tag="o", name=f"o{h}_{i}")
                        for i in range(NTQB)
                    ]
                    pending = None
                    et = None
                    for tkc in range(NTK):
                        if tkc % 2 == 0:
                            et = epool.tile([128, 2, T], FP16, tag="e")
                            nc.gpsimd.dma_start(
                                et[:],
                                d_E.ap()[h][
                                    tkc * 128 : (tkc + 2) * 128, :
                                ].rearrange("(a p) t -> p a t", a=2),
                            )
                        cur = []
                        for tqb in range(NTQB):
                            spt = sps.tile([128, TQB], F32, tag="s")
                            for q2 in range(TQB // 512):
                                nc.tensor.matmul(
                                    spt[:, ts(q2, 512)],
                                    lhsT=kt_sb[hp : hp + 64, j, ts(tkc, 128)],
                                    rhs=qt_sb[
                                        hp : hp + 64, j, tqb * TQB + q2 * 512 :
                                        tqb * TQB + (q2 + 1) * 512
                                    ],
                                    start=True,
                                    stop=True,
                                )
                            ex = xpool.tile([128, TQB], FP16, tag="ex")
                            nc.scalar.activation(ex[:], spt[:], Exp)
                            pp = xpool.tile([128, TQB], FP16, tag="pp")
                            nc.vector.tensor_mul(
                                pp[:], ex[:], et[:, tkc % 2, ts(tqb, TQB)]
                            )
                            cur.append(pp)
                        if pending is not None:
                            _avs(h, otiles, tkc - 1, pending)
                        pending = cur
                    _avs(h, otiles, NTK - 1, pending)

                    # denom lands on psum row 0 (ones column is first in
                    # the AV lhsT); dh rows start at partition 32. Everything
                    # quadrant-aligned: no partition shifts, no ACT involved.
                    for tqb in range(NTQB):
                        tq_sl = slice(tqb * TQB, (tqb + 1) * TQB)
                        rc = npool.tile([1, TQB], F32, tag="rc")
                        nc.vector.reciprocal_approx_fast(
                            rc[:], otiles[tqb][0:1, :]
                        )
                        rep = npool.tile([64, TQB], F32, tag="rep")
                        nc.gpsimd.partition_broadcast(rep[:], rc[:])
                        nc.vector.tensor_mul(
                            ot_sb[hp : hp + 64, j, tq_sl],
                            otiles[tqb][64 : 64 + DH, :],
                            rep[:],
                        )

        def _stage4():
            with (
                tc.tile_pool(name="pops", bufs=4, space="PSUM") as pops,
                tc.tile_pool(name="outst", bufs=2) as outst,
            ):
                for trp in range(T // 256):
                    ost = outst.tile([128, 2, DM], FP16, tag="ost")
                    for a in range(2):
                        tr = trp * 2 + a
                        for dmc in range(DM // 1024):
                            po = pops.tile([128, 1024], F32, tag="po")
                            for j in range(NJ):
                                for q2 in range(2):
                                    nc.tensor.matmul(
                                        po[:, ts(q2, 512)],
                                        lhsT=ot_sb[:, j, ts(tr, 128)],
                                        rhs=wo_sb[
                                            :, j,
                                            dmc * 1024 + q2 * 512 :
                                            dmc * 1024 + (q2 + 1) * 512,
                                        ],
                                        start=(j == 0),
                                        stop=(j == NJ - 1),
                                    )
                            if dmc % 2 == 0:
                                nc.scalar.activation(
                                    ost[:, a, ts(dmc, 1024)], po[:], Copy
                                )
                            else:
                                nc.vector.tensor_copy(
                                    ost[:, a, ts(dmc, 1024)], po[:]
                                )
                    nc.scalar.dma_start(
                        d_out.ap()[ts(trp, 256), :].rearrange(
                            "(a p) d -> p a d", a=2
                        ),
                        ost[:],
                    )

        def _compute_body():
            with tc.tile_pool(name="hin", bufs=3) as hin:
                _proj_kv_fused()
                _proj_pass(d_hqT, wq_sb, bq_sb, qt_sb, hin)
            _transpose_v()
            _stage23()
            _stage4()

        if reps > 1:
            hints = (
                mybir.EngineType.PE,
                mybir.EngineType.Activation,
                mybir.EngineType.DVE,
                mybir.EngineType.SP,
                mybir.EngineType.Pool,
            )
            with tc.For_i(0, reps, 1, hint_engines=hints):
                _compute_body()
        else:
            _compute_body()

    nc.compile()
    return nc


def _pack_w(wT, ndc):
    """[DM, m] (contraction-major) -> [128, ndc*m] SBUF-layout prepack."""
    dm, m = wT.shape
    return np.ascontiguousarray(
        wT.reshape(ndc, 128, m).transpose(1, 0, 2).reshape(128, ndc * m)
    )


def make_in_maps(hidden_q, hidden_kv, mask, position_bias, wq, bq, wk, bk, wv, wo):
    f16 = np.float16
    T = hidden_q.shape[1]
    DM = hidden_q.shape[2]
    NDC = DM // 128
    NJ = M // 128
    hqT = np.ascontiguousarray(hidden_q[0].T, dtype=f16)
    hkvT = np.ascontiguousarray(hidden_kv[0].T, dtype=f16)
    maskf = mask[0].astype(np.float32)
    in_maps = []
    for c in range(N_CORES):
        sl = slice(c * M, (c + 1) * M)
        wqT = (wq[sl] * (1.0 / np.sqrt(DH))).T.astype(f16)
        wkT = wk[sl].T.astype(f16)
        wvT = wv[sl].T.astype(f16)
        woT = wo[:, sl].T.astype(f16)
        pb_c = position_bias[0, c * HPC : (c + 1) * HPC]
        E = np.exp(pb_c, dtype=np.float32) * (maskf[None] * E_SCALE)
        E = np.ascontiguousarray(E.transpose(0, 2, 1)).astype(f16)
        in_maps.append(
            {
                "hqT": hqT,
                "hkvT": hkvT,
                "wqT": _pack_w(wqT, NDC),
                "wkT": _pack_w(wkT, NDC),
                "wvT": _pack_w(wvT, NDC),
                "woT": _pack_w(woT, NJ),
                "bq": np.ascontiguousarray(bq[sl] * (1.0 / np.sqrt(DH))),
                "bk": np.ascontiguousarray(bk[sl]),
                "E": E,
            }
        )
    return in_maps


def kernel(hidden_q, hidden_kv, mask, position_bias, wq, bq, wk, bk, wv, bv, wo, bo):
    hidden_q = np.asarray(hidden_q, np.float32)
    hidden_kv = np.asarray(hidden_kv, np.float32)
    mask = np.asarray(mask)
    position_bias = np.asarray(position_bias, np.float32)
    wq, bq = np.asarray(wq, np.float32), np.asarray(bq, np.float32)
    wk, bk = np.asarray(wk, np.float32), np.asarray(bk, np.float32)
    wv, bv = np.asarray(wv, np.float32), np.asarray(bv, np.float32)
    wo, bo = np.asarray(wo, np.float32), np.asarray(bo, np.float32)

    T = hidden_q.shape[1]
    DM = hidden_q.shape[2]

    with_bias = bool(np.any(bq) or np.any(bk))
    nc = build_attention_nc(T, DM, with_bias=with_bias)
    in_maps = make_in_maps(
        hidden_q, hidden_kv, mask, position_bias, wq, bq, wk, bk, wv, wo
    )
    res = run_bass_kernel_spmd(nc, in_maps, core_ids=list(range(N_CORES)))
    if res.exec_time_ns is not None:
        print(f"HW exec time: {res.exec_time_ns} ns")

    out = np.zeros((T, DM), np.float64)
    for c in range(N_CORES):
        out += res.results[c]["out"].astype(np.float64)
    out += (bv.astype(np.float64) @ wo.T.astype(np.float64)) + bo.astype(np.float64)
    return out[None].astype(np.float32)


# revision 6
# speedup vs baseline: 3.4464x; 3.4464x over previous
"""Trainium2 Bass kernel for multi-head attention (nn_Attention_39573828665669).

Reference computation (per batch b=1, SQ=SKV=2048, DM=2048, H=32, DH=64):
    q = hidden_q @ wq.T + bq ; k = hidden_kv @ wk.T + bk ; v = hidden_kv @ wv.T + bv
    score = q @ k.T / sqrt(DH) + position_bias ; masked softmax ; out = probs @ v
    return out @ wo.T + bo

Sharding: tensor-parallel over heads. Each of the 8 cores handles 4 heads
(256 of the 2048 head dims), computes its partial output projection
out_c = attn_c @ wo[:, cols_c].T, and the host sums the 8 partials.

Device-side formulation (per core), all matmuls fp16 with fp32 psum:
  - 1/sqrt(DH) folded into wq on the host; mask and position_bias folded
    into E = exp(pb) * mask / 256 (fp16, [tk, tq] per head). The device
    computes P' = exp(S) * E unnormalized; a leading ones-column in the
    V block makes the AV matmul emit the softmax denominator on psum
    partition 0 (per-head AV block layout: [1 ones | 63 zero | 64 dh],
    which keeps every later access quadrant-aligned).
  - 1/denom via DVE reciprocal_approx_fast at partition 0 + Pool
    partition_broadcast; normalization fuses into the psum evacuation
    multiply on DVE. No ACT table swaps anywhere (exp/copy share a table).
  - bv is folded on the host as out += bv @ wo.T; bo added on the host.

Schedule, tuned against the TimelineSim cost model:
  - K and V^T share one streaming pass over hkvT (half-T psum split:
    4 banks K + 4 banks V^T); V^T is then transposed to [t, m] with 32
    [128,128] PE transposes through fp16 psum. Q uses a full-T pass.
    All projection passes are 512-wide-rhs, d-chunk-outer, full-psum.
  - DMA is batched 2 chunks per trigger (8 KB per partition line) and
    spread across queues: hq/hkv on SP, weights+E on Pool, out on ACT.
    E prefetches during stage 1; hq prefetches during the KV pass.
  - Stage 2/3 per head: S (2x512-wide per tkc/tqb) -> ACT exp ->
    DVE mul by E -> AV, with AV lagging one tkc so the PE never waits
    on the exp/mul chain. ACT runs ~100% busy here and is the pacer.
  - Output projection accumulates [128,1024] psum tiles (2 banks, 4
    matmuls) with ACT/DVE-alternating evacuation and [128,2,DM] stores.
"""

import os
from contextlib import ExitStack

import numpy as np

import concourse.bass as bass
import concourse.tile as tile
from concourse import bacc, mybir
from concourse.bass_utils import run_bass_kernel_spmd
from concourse.masks import make_identity

F32 = mybir.dt.float32
FP16 = mybir.dt.float16

ts = bass.ts

N_CORES = 8
H = 32
DH = 64
HPC = H // N_CORES          # heads per core = 4
M = HPC * DH                # per-core head dims = 256
E_SCALE = 1.0 / 256.0


def build_attention_nc(T: int, DM: int, reps: int = 1, with_bias: bool = False):
    assert HPC == 4 and DH == 64
    NJ = M // 128            # m chunks (2)
    NDC = DM // 128          # contraction chunks for projections
    NTK = T // 128           # key-position chunks
    NTQ = T // 512           # 512-wide query chunks
    TQB = min(1024, T)
    NTQB = T // TQB

    nc = bacc.Bacc("TRN2", target_bir_lowering=False, debug=False)

    d_hqT = nc.declare_dram_parameter("hqT", [DM, T], FP16, isOutput=False)
    d_hkvT = nc.declare_dram_parameter("hkvT", [DM, T], FP16, isOutput=False)
    d_wq = nc.declare_dram_parameter("wqT", [128, NDC * M], FP16, isOutput=False)
    d_wk = nc.declare_dram_parameter("wkT", [128, NDC * M], FP16, isOutput=False)
    d_wv = nc.declare_dram_parameter("wvT", [128, NDC * M], FP16, isOutput=False)
    d_wo = nc.declare_dram_parameter("woT", [128, NJ * DM], FP16, isOutput=False)
    d_bq = nc.declare_dram_parameter("bq", [M], F32, isOutput=False)
    d_bk = nc.declare_dram_parameter("bk", [M], F32, isOutput=False)
    d_E = nc.declare_dram_parameter("E", [HPC, T, T], FP16, isOutput=False)
    d_out = nc.declare_dram_parameter("out", [T, DM], FP16, isOutput=True)

    Exp = mybir.ActivationFunctionType.Exp
    Ln = mybir.ActivationFunctionType.Ln
    Copy = mybir.ActivationFunctionType.Copy
    Identity = mybir.ActivationFunctionType.Identity

    with tile.TileContext(nc) as tc, ExitStack() as ctx:
        wpool = ctx.enter_context(tc.tile_pool(name="weights", bufs=1))
        spool = ctx.enter_context(tc.tile_pool(name="state", bufs=1))

        wq_sb = wpool.tile([128, NDC, M], FP16, tag="wq")
        wk_sb = wpool.tile([128, NDC, M], FP16, tag="wk")
        wv_sb = wpool.tile([128, NDC, M], FP16, tag="wv")
        wo_sb = wpool.tile([128, NJ, DM], FP16, tag="wo")
        bq_sb = wpool.tile([128, NJ], F32, tag="bq")
        bk_sb = wpool.tile([128, NJ], F32, tag="bk")
        id_sb = wpool.tile([128, 128], FP16, tag="idn")
        # weights on the Pool queue so they overlap the first hq tiles (SP)
        nc.gpsimd.dma_start(
            wk_sb[:, 0:1, :].rearrange("p a b -> p (a b)"), d_wk.ap()[:, 0:M]
        )
        nc.gpsimd.dma_start(
            wv_sb[:, 0:1, :].rearrange("p a b -> p (a b)"), d_wv.ap()[:, 0:M]
        )
        nc.gpsimd.dma_start(
            wk_sb[:, 1:, :].rearrange("p a b -> p (a b)"), d_wk.ap()[:, M:]
        )
        nc.gpsimd.dma_start(
            wv_sb[:, 1:, :].rearrange("p a b -> p (a b)"), d_wv.ap()[:, M:]
        )
        nc.gpsimd.dma_start(wq_sb[:].rearrange("p a b -> p (a b)"), d_wq.ap())
        nc.gpsimd.dma_start(wo_sb[:].rearrange("p a b -> p (a b)"), d_wo.ap())
        nc.sync.dma_start(bq_sb[:], d_bq.ap().rearrange("(j p) -> p j", p=128))
        nc.sync.dma_start(bk_sb[:], d_bk.ap().rearrange("(j p) -> p j", p=128))
        make_identity(nc, id_sb[:])

        qt_sb = spool.tile([128, NJ, T], FP16, tag="qt")     # Q.T / sqrt(DH)
        kt_sb = spool.tile([128, NJ, T], FP16, tag="kt")     # K.T
        vt_sb = spool.tile([128, NJ, T], FP16, tag="vt")     # V.T (pre-transpose)
        VW = 128  # per-head AV lhsT width: [1 ones | 63 pad | 64 dh]
        vo_sb = spool.tile([128, NTK, HPC * VW], FP16, tag="vo")
        ot_sb = spool.tile([128, NJ, T], FP16, tag="ot")     # normalized O.T

        nc.vector.memset(vo_sb[:], 0.0)
        ones_view = vo_sb[:].rearrange("p n (h x) -> p n h x", x=VW)[:, :, :, 0:1]
        nc.vector.memset(ones_view, 1.0)

        def _evac(dst, src, b_sb, j, idx):
            if with_bias and b_sb is not None:
                nc.scalar.activation(dst, src, Identity, bias=b_sb[:, j : j + 1])
            elif idx % 2 == 0:
                nc.scalar.activation(dst, src, Copy)
            else:
                nc.vector.tensor_copy(dst, src)

        def _proj_pass(d_src, w_sb, b_sb, dst_sb, hin):
            # d-chunk-outer projection: full-T psum (NJ * NTQ = 8 banks),
            # one [128, 2, T] rhs DMA per dc pair (8 KB per partition line).
            with tc.tile_pool(name="pqk", bufs=8, space="PSUM") as pqk:
                pp = [
                    pqk.tile([128, 512], F32, tag="pp", name=f"pp{i}")
                    for i in range(NJ * NTQ)
                ]
                for dcp in range(NDC // 2):
                    ht = hin.tile([128, 2, T], FP16, tag="h")
                    nc.sync.dma_start(
                        ht[:],
                        d_src.ap()[ts(dcp, 256), :].rearrange(
                            "(a p) t -> p a t", a=2
                        ),
                    )
                    for a in range(2):
                        dc = dcp * 2 + a
                        for j in range(NJ):
                            for q in range(NTQ):
                                nc.tensor.matmul(
                                    pp[j * NTQ + q][:],
                                    lhsT=w_sb[:, dc, ts(j, 128)],
                                    rhs=ht[:, a, ts(q, 512)],
                                    start=(dc == 0),
                                    stop=(dc == NDC - 1),
                                )
                for j in range(NJ):
                    for q in range(NTQ):
                        _evac(
                            dst_sb[:, j, ts(q, 512)],
                            pp[j * NTQ + q][:],
                            b_sb,
                            j,
                            j * NTQ + q,
                        )

        def _proj_kv_fused():
            # K and V^T share one streaming pass over hkvT: half-T psum per
            # projection (4 banks K + 4 banks V^T), two column-halves.
            TH = T // 2
            for half in range(2):
                with (
                    tc.tile_pool(name="pkv", bufs=4, space="PSUM") as pkv,
                    tc.tile_pool(name="hin2", bufs=3) as hin2,
                ):
                    pk = [
                        pkv.tile([128, 512], F32, tag="pk", name=f"pk{half}_{i}")
                        for i in range(NJ * 2)
                    ]
                    pv = [
                        pkv.tile([128, 512], F32, tag="pv", name=f"pv{half}_{i}")
                        for i in range(NJ * 2)
                    ]
                    for dcp in range(NDC // 2):
                        ht = hin2.tile([128, 2, TH], FP16, tag="h2")
                        nc.sync.dma_start(
                            ht[:],
                            d_hkvT.ap()[
                                ts(dcp, 256), half * TH : (half + 1) * TH
                            ].rearrange("(a p) t -> p a t", a=2),
                        )
                        for a in range(2):
                            dc = dcp * 2 + a
                            for j in range(NJ):
                                for q in range(2):
                                    nc.tensor.matmul(
                                        pk[j * 2 + q][:],
                                        lhsT=wk_sb[:, dc, ts(j, 128)],
                                        rhs=ht[:, a, ts(q, 512)],
                                        start=(dc == 0),
                                        stop=(dc == NDC - 1),
                                    )
                                    nc.tensor.matmul(
                                        pv[j * 2 + q][:],
                                        lhsT=wv_sb[:, dc, ts(j, 128)],
                                        rhs=ht[:, a, ts(q, 512)],
                                        start=(dc == 0),
                                        stop=(dc == NDC - 1),
                                    )
                    for j in range(NJ):
                        for q in range(2):
                            col = half * TH + q * 512
                            _evac(
                                kt_sb[:, j, col : col + 512],
                                pk[j * 2 + q][:],
                                bk_sb,
                                j,
                                j * 2 + q,
                            )
                            _evac(
                                vt_sb[:, j, col : col + 512],
                                pv[j * 2 + q][:],
                                None,
                                j,
                                j * 2 + q + 1,
                            )

        def _transpose_v():
            # vt [m, t] -> vo [t, m] in [128,128] blocks on the PE.
            with tc.tile_pool(name="ptr", bufs=4, space="PSUM") as ptr:
                for tkc in range(NTK):
                    for j in range(NJ):
                        pt = ptr.tile([128, 128], FP16, tag="pt")
                        nc.tensor.transpose(
                            pt[:], vt_sb[:, j, ts(tkc, 128)], id_sb[:]
                        )
                        dst = vo_sb[:, tkc, :].rearrange(
                            "p (h x) -> p h x", x=VW
                        )[:, 2 * j : 2 * j + 2, 64 : 64 + DH]
                        nc.vector.tensor_copy(
                            dst, pt[:].rearrange("p (h d) -> p h d", d=DH)
                        )

        def _stage23():
            with (
                tc.tile_pool(name="sps", bufs=2, space="PSUM") as sps,
                tc.tile_pool(name="ops", bufs=2, space="PSUM") as ops,
                tc.tile_pool(name="epool", bufs=4) as epool,
                tc.tile_pool(name="xpool", bufs=4) as xpool,
                tc.tile_pool(name="npool", bufs=2) as npool,
            ):

                def _avs(h, otiles, tkc, pps):
                    for tqb in range(NTQB):
                        for q2 in range(TQB // 512):
                            nc.tensor.matmul(
                                otiles[tqb][:, ts(q2, 512)],
                                lhsT=vo_sb[:, tkc, h * VW : (h + 1) * VW],
                                rhs=pps[tqb][:, ts(q2, 512)],
                                start=(tkc == 0),
                                stop=(tkc == NTK - 1),
                            )

                for h in range(HPC):
                    j, hp = h // 2, 64 * (h % 2)
                    otiles = [
                        ops.tile([128, TQB], F32, tag="o", name=f"o{h}_{i}")
                        for i in range(NTQB)
                    ]
                    pending = None
                    et = None
                    for tkc in range(NTK):
                        if tkc % 2 == 0:
                            et = epool.tile([128, 2, T], FP16, tag="e")
                            nc.gpsimd.dma_start(
                                et[:],
                                d_E.ap()[h][
                                    tkc * 128 : (tkc + 2) * 128, :
                                ].rearrange("(a p) t -> p a t", a=2),
                            )
                        cur = []
                        for tqb in range(NTQB):
                            spt = sps.tile([128, TQB], F32, tag="s")
                            for q2 in range(TQB // 512):
                                nc.tensor.matmul(
                                    spt[:, ts(q2, 512)],
                                    lhsT=kt_sb[hp : hp + 64, j, ts(tkc, 128)],
                                    rhs=qt_sb[
                                        hp : hp + 64, j, tqb * TQB + q2 * 512 :
                                        tqb * TQB + (q2 + 1) * 512
                                    ],
                                    start=True,
                                    stop=True,
                                )
                            ex = xpool.tile([128, TQB], FP16, tag="ex")
                            nc.scalar.activation(ex[:], spt[:], Exp)
                            pp = xpool.tile([128, TQB], FP16, tag="pp")
                            nc.vector.tensor_mul(
                                pp[:], ex[:], et[:, tkc % 2, ts(tqb, TQB)]
                            )
                            cur.append(pp)
                        if pending is not None:
                            _avs(h, otiles, tkc - 1, pending)
                        pending = cur
                    _avs(h, otiles, NTK - 1, pending)

                    # denom lands on psum row 0 (ones column is first in
                    # the AV lhsT); dh rows start at partition 32. Everything
                    # quadrant-aligned: no partition shifts, no ACT involved.
                    for tqb in range(NTQB):
                        tq_sl = slice(tqb * TQB, (tqb + 1) * TQB)
                        rc = npool.tile([1, TQB], F32, tag="rc")
                        nc.vector.reciprocal_approx_fast(
                            rc[:], otiles[tqb][0:1, :]
                        )
                        rep = npool.tile([64, TQB], F32, tag="rep")
                        nc.gpsimd.partition_broadcast(rep[:], rc[:])
                        nc.vector.tensor_mul(
                            ot_sb[hp : hp + 64, j, tq_sl],
                            otiles[tqb][64 : 64 + DH, :],
                            rep[:],
                        )

        def _stage4():
            with (
                tc.tile_pool(name="pops", bufs=4, space="PSUM") as pops,
                tc.tile_pool(name="outst", bufs=2) as outst,
            ):
                for trp in range(T // 256):
                    ost = outst.tile([128, 2, DM], FP16, tag="ost")
                    for a in range(2):
                        tr = trp * 2 + a
                        for dmc in range(DM // 1024):
                            po = pops.tile([128, 1024], F32, tag="po")
                            for j in range(NJ):
                                for q2 in range(2):
                                    nc.tensor.matmul(
                                        po[:, ts(q2, 512)],
                                        lhsT=ot_sb[:, j, ts(tr, 128)],
                                        rhs=wo_sb[
                                            :, j,
                                            dmc * 1024 + q2 * 512 :
                                            dmc * 1024 + (q2 + 1) * 512,
                                        ],
                                        start=(j == 0),
                                        stop=(j == NJ - 1),
                                    )
                            if dmc % 2 == 0:
                                nc.scalar.activation(
                                    ost[:, a, ts(dmc, 1024)], po[:], Copy
                                )
                            else:
                                nc.vector.tensor_copy(
                                    ost[:, a, ts(dmc, 1024)], po[:]
                                )
                    nc.scalar.dma_start(
                        d_out.ap()[ts(trp, 256), :].rearrange(
                            "(a p) d -> p a d", a=2
                        ),
                        ost[:],
                    )

        def _compute_body():
            with tc.tile_pool(name="hin", bufs=3) as hin:
                _proj_kv_fused()
                _proj_pass(d_hqT, wq_sb, bq_sb, qt_sb, hin)
            _transpose_v()
            _stage23()
            _stage4()

        if reps > 1:
            hints = (
                mybir.EngineType.PE,
                mybir.EngineType.Activation,
                mybir.EngineType.DVE,
                mybir.EngineType.SP,
                mybir.EngineType.Pool,
            )
            with tc.For_i(0, reps, 1, hint_engines=hints):
                _compute_body()
        else:
            _compute_body()

    nc.compile()
    return nc


def _pack_w(wT, ndc):
    """[DM, m] (contraction-major) -> [128, ndc*m] SBUF-layout prepack."""
    dm, m = wT.shape
    return np.ascontiguousarray(
        wT.reshape(ndc, 128, m).transpose(1, 0, 2).reshape(128, ndc * m)
    )


def make_in_maps(hidden_q, hidden_kv, mask, position_bias, wq, bq, wk, bk, wv, wo):
    f16 = np.float16
    T = hidden_q.shape[1]
    DM = hidden_q.shape[2]
    NDC = DM // 128
    NJ = M // 128
    hqT = np.ascontiguousarray(hidden_q[0].T, dtype=f16)
    hkvT = np.ascontiguousarray(hidden_kv[0].T, dtype=f16)
    maskf = mask[0].astype(np.float32)
    in_maps = []
    for c in range(N_CORES):
        sl = slice(c * M, (c + 1) * M)
        wqT = (wq[sl] * (1.0 / np.sqrt(DH))).T.astype(f16)
        wkT = wk[sl].T.astype(f16)
        wvT = wv[sl].T.astype(f16)
        woT = wo[:, sl].T.astype(f16)
        pb_c = position_bias[0, c * HPC : (c + 1) * HPC]
        E = np.exp(pb_c, dtype=np.float32) * (maskf[None] * E_SCALE)
        E = np.ascontiguousarray(E.transpose(0, 2, 1)).astype(f16)
        in_maps.append(
            {
                "hqT": hqT,
                "hkvT": hkvT,
                "wqT": _pack_w(wqT, NDC),
                "wkT": _pack_w(wkT, NDC),
                "wvT": _pack_w(wvT, NDC),
                "woT": _pack_w(woT, NJ),
                "bq": np.ascontiguousarray(bq[sl] * (1.0 / np.sqrt(DH))),
                "bk": np.ascontiguousarray(bk[sl]),
                "E": E,
            }
        )
    return in_maps


def kernel(hidden_q, hidden_kv, mask, position_bias, wq, bq, wk, bk, wv, bv, wo, bo):
    hidden_q = np.asarray(hidden_q, np.float32)
    hidden_kv = np.asarray(hidden_kv, np.float32)
    mask = np.asarray(mask)
    position_bias = np.asarray(position_bias, np.float32)
    wq, bq = np.asarray(wq, np.float32), np.asarray(bq, np.float32)
    wk, bk = np.asarray(wk, np.float32), np.asarray(bk, np.float32)
    wv, bv = np.asarray(wv, np.float32), np.asarray(bv, np.float32)
    wo, bo = np.asarray(wo, np.float32), np.asarray(bo, np.float32)

    T = hidden_q.shape[1]
    DM = hidden_q.shape[2]

    with_bias = bool(np.any(bq) or np.any(bk))
    nc = build_attention_nc(T, DM, with_bias=with_bias)
    in_maps = make_in_maps(
        hidden_q, hidden_kv, mask, position_bias, wq, bq, wk, bk, wv, wo
    )
    res = run_bass_kernel_spmd(nc, in_maps, core_ids=list(range(N_CORES)))
    if res.exec_time_ns is not None:
        print(f"HW exec time: {res.exec_time_ns} ns")

    out = np.zeros((T, DM), np.float64)
    for c in range(N_CORES):
        out += res.results[c]["out"].astype(np.float64)
    out += (bv.astype(np.float64) @ wo.T.astype(np.float64)) + bo.astype(np.float64)
    return out[None].astype(np.float32)


# revision 7
# speedup vs baseline: 3.4623x; 1.0046x over previous
"""Trainium2 Bass kernel for multi-head attention (nn_Attention_39573828665669).

Reference computation (per batch b=1, SQ=SKV=2048, DM=2048, H=32, DH=64):
    q = hidden_q @ wq.T + bq ; k = hidden_kv @ wk.T + bk ; v = hidden_kv @ wv.T + bv
    score = q @ k.T / sqrt(DH) + position_bias ; masked softmax ; out = probs @ v
    return out @ wo.T + bo

Sharding: tensor-parallel over heads. Each of the 8 cores handles 4 heads
(256 of the 2048 head dims), computes its partial output projection
out_c = attn_c @ wo[:, cols_c].T, and the host sums the 8 partials.

Device-side formulation (per core), all matmuls fp16 with fp32 psum:
  - 1/sqrt(DH) folded into wq on the host; mask and position_bias folded
    into E = exp(pb) * mask / 256 (fp16, [tk, tq] per head). The device
    computes P' = exp(S) * E unnormalized; a leading ones-column in the
    V block makes the AV matmul emit the softmax denominator on psum
    partition 0 (per-head AV block layout: [1 ones | 63 zero | 64 dh],
    which keeps every later access quadrant-aligned).
  - 1/denom via DVE reciprocal_approx_fast at partition 0 + Pool
    partition_broadcast; normalization fuses into the psum evacuation
    multiply on DVE. No ACT table swaps anywhere (exp/copy share a table).
  - bv is folded on the host as out += bv @ wo.T; bo added on the host.

Schedule, tuned against the TimelineSim cost model:
  - K and V^T share one streaming pass over hkvT (half-T psum split:
    4 banks K + 4 banks V^T); V^T is then transposed to [t, m] with 32
    [128,128] PE transposes through fp16 psum. Q uses a full-T pass.
    All projection passes are 512-wide-rhs, d-chunk-outer, full-psum.
  - DMA is batched 2 chunks per trigger (8 KB per partition line) and
    spread across queues: hq/hkv on SP, weights+E on Pool, out on ACT.
    E prefetches during stage 1; hq prefetches during the KV pass.
  - Stage 2/3 per head: S (2x512-wide per tkc/tqb) -> ACT exp ->
    DVE mul by E -> AV, with AV lagging one tkc so the PE never waits
    on the exp/mul chain. ACT runs ~100% busy here and is the pacer.
  - Output projection accumulates [128,1024] psum tiles (2 banks, 4
    matmuls) with ACT/DVE-alternating evacuation and [128,2,DM] stores.
"""

import os
from contextlib import ExitStack

import numpy as np

import concourse.bass as bass
import concourse.tile as tile
from concourse import bacc, mybir
from concourse.bass_utils import run_bass_kernel_spmd
from concourse.masks import make_identity

F32 = mybir.dt.float32
FP16 = mybir.dt.float16

ts = bass.ts

N_CORES = 8
H = 32
DH = 64
HPC = H // N_CORES          # heads per core = 4
M = HPC * DH                # per-core head dims = 256
E_SCALE = 1.0 / 256.0


def build_attention_nc(T: int, DM: int, reps: int = 1, with_bias: bool = False):
    assert HPC == 4 and DH == 64
    NJ = M // 128            # m chunks (2)
    NDC = DM // 128          # contraction chunks for projections
    NTK = T // 128           # key-position chunks
    NTQ = T // 512           # 512-wide query chunks
    TQB = min(1024, T)
    NTQB = T // TQB

    nc = bacc.Bacc("TRN2", target_bir_lowering=False, debug=False)

    d_hqT = nc.declare_dram_parameter("hqT", [DM, T], FP16, isOutput=False)
    d_hkvT = nc.declare_dram_parameter("hkvT", [DM, T], FP16, isOutput=False)
    d_wq = nc.declare_dram_parameter("wqT", [128, NDC * M], FP16, isOutput=False)
    d_wk = nc.declare_dram_parameter("wkT", [128, NDC * M], FP16, isOutput=False)
    d_wv = nc.declare_dram_parameter("wvT", [128, NDC * M], FP16, isOutput=False)
    d_wo = nc.declare_dram_parameter("woT", [128, NJ * DM], FP16, isOutput=False)
    d_bq = nc.declare_dram_parameter("bq", [M], F32, isOutput=False)
    d_bk = nc.declare_dram_parameter("bk", [M], F32, isOutput=False)
    d_E = nc.declare_dram_parameter("E", [HPC, T, T], FP16, isOutput=False)
    d_out = nc.declare_dram_parameter("out", [T, DM], FP16, isOutput=True)

    Exp = mybir.ActivationFunctionType.Exp
    Ln = mybir.ActivationFunctionType.Ln
    Copy = mybir.ActivationFunctionType.Copy
    Identity = mybir.ActivationFunctionType.Identity

    with tile.TileContext(nc) as tc, ExitStack() as ctx:
        wpool = ctx.enter_context(tc.tile_pool(name="weights", bufs=1))
        spool = ctx.enter_context(tc.tile_pool(name="state", bufs=1))

        wq_sb = wpool.tile([128, NDC, M], FP16, tag="wq")
        wk_sb = wpool.tile([128, NDC, M], FP16, tag="wk")
        wv_sb = wpool.tile([128, NDC, M], FP16, tag="wv")
        wo_sb = wpool.tile([128, NJ, DM], FP16, tag="wo")
        bq_sb = wpool.tile([128, NJ], F32, tag="bq")
        bk_sb = wpool.tile([128, NJ], F32, tag="bk")
        id_sb = wpool.tile([128, 128], FP16, tag="idn")
        # weights on the Pool queue so they overlap the first hq tiles (SP)
        nc.gpsimd.dma_start(
            wk_sb[:, 0:1, :].rearrange("p a b -> p (a b)"), d_wk.ap()[:, 0:M]
        )
        nc.gpsimd.dma_start(
            wv_sb[:, 0:1, :].rearrange("p a b -> p (a b)"), d_wv.ap()[:, 0:M]
        )
        nc.gpsimd.dma_start(
            wk_sb[:, 1:, :].rearrange("p a b -> p (a b)"), d_wk.ap()[:, M:]
        )
        nc.gpsimd.dma_start(
            wv_sb[:, 1:, :].rearrange("p a b -> p (a b)"), d_wv.ap()[:, M:]
        )
        nc.gpsimd.dma_start(wq_sb[:].rearrange("p a b -> p (a b)"), d_wq.ap())
        nc.gpsimd.dma_start(wo_sb[:].rearrange("p a b -> p (a b)"), d_wo.ap())
        nc.sync.dma_start(bq_sb[:], d_bq.ap().rearrange("(j p) -> p j", p=128))
        nc.sync.dma_start(bk_sb[:], d_bk.ap().rearrange("(j p) -> p j", p=128))
        make_identity(nc, id_sb[:])

        qt_sb = spool.tile([128, NJ, T], FP16, tag="qt")     # Q.T / sqrt(DH)
        kt_sb = spool.tile([128, NJ, T], FP16, tag="kt")     # K.T
        vt_sb = spool.tile([128, NJ, T], FP16, tag="vt")     # V.T (pre-transpose)
        VW = 128  # per-head AV lhsT width: [1 ones | 63 pad | 64 dh]
        vo_sb = spool.tile([128, NTK, HPC * VW], FP16, tag="vo")
        ot_sb = spool.tile([128, NJ, T], FP16, tag="ot")     # normalized O.T

        nc.vector.memset(vo_sb[:], 0.0)
        ones_view = vo_sb[:].rearrange("p n (h x) -> p n h x", x=VW)[:, :, :, 0:1]
        nc.vector.memset(ones_view, 1.0)

        def _evac(dst, src, b_sb, j, idx):
            if with_bias and b_sb is not None:
                nc.scalar.activation(dst, src, Identity, bias=b_sb[:, j : j + 1])
            elif idx % 2 == 0:
                nc.scalar.activation(dst, src, Copy)
            else:
                nc.vector.tensor_copy(dst, src)

        def _proj_pass(d_src, w_sb, b_sb, dst_sb, hin):
            # d-chunk-outer projection: full-T psum (NJ * NTQ = 8 banks),
            # one [128, 2, T] rhs DMA per dc pair (8 KB per partition line).
            with tc.tile_pool(name="pqk", bufs=8, space="PSUM") as pqk:
                pp = [
                    pqk.tile([128, 512], F32, tag="pp", name=f"pp{i}")
                    for i in range(NJ * NTQ)
                ]
                for dcp in range(NDC // 2):
                    ht = hin.tile([128, 2, T], FP16, tag="h")
                    nc.sync.dma_start(
                        ht[:],
                        d_src.ap()[ts(dcp, 256), :].rearrange(
                            "(a p) t -> p a t", a=2
                        ),
                    )
                    for a in range(2):
                        dc = dcp * 2 + a
                        for j in range(NJ):
                            for q in range(NTQ):
                                nc.tensor.matmul(
                                    pp[j * NTQ + q][:],
                                    lhsT=w_sb[:, dc, ts(j, 128)],
                                    rhs=ht[:, a, ts(q, 512)],
                                    start=(dc == 0),
                                    stop=(dc == NDC - 1),
                                )
                for j in range(NJ):
                    for q in range(NTQ):
                        _evac(
                            dst_sb[:, j, ts(q, 512)],
                            pp[j * NTQ + q][:],
                            b_sb,
                            j,
                            j * NTQ + q,
                        )

        def _proj_kv_fused():
            # K and V^T share one streaming pass over hkvT: half-T psum per
            # projection (4 banks K + 4 banks V^T), two column-halves.
            TH = T // 2
            for half in range(2):
                with (
                    tc.tile_pool(name="pkv", bufs=4, space="PSUM") as pkv,
                    tc.tile_pool(name="hin2", bufs=3) as hin2,
                ):
                    pk = [
                        pkv.tile([128, 512], F32, tag="pk", name=f"pk{half}_{i}")
                        for i in range(NJ * 2)
                    ]
                    pv = [
                        pkv.tile([128, 512], F32, tag="pv", name=f"pv{half}_{i}")
                        for i in range(NJ * 2)
                    ]
                    for dcp in range(NDC // 2):
                        ht = hin2.tile([128, 2, TH], FP16, tag="h2")
                        nc.sync.dma_start(
                            ht[:],
                            d_hkvT.ap()[
                                ts(dcp, 256), half * TH : (half + 1) * TH
                            ].rearrange("(a p) t -> p a t", a=2),
                        )
                        for a in range(2):
                            dc = dcp * 2 + a
                            for j in range(NJ):
                                for q in range(2):
                                    nc.tensor.matmul(
                                        pk[j * 2 + q][:],
                                        lhsT=wk_sb[:, dc, ts(j, 128)],
                                        rhs=ht[:, a, ts(q, 512)],
                                        start=(dc == 0),
                                        stop=(dc == NDC - 1),
                                    )
                                    nc.tensor.matmul(
                                        pv[j * 2 + q][:],
                                        lhsT=wv_sb[:, dc, ts(j, 128)],
                                        rhs=ht[:, a, ts(q, 512)],
                                        start=(dc == 0),
                                        stop=(dc == NDC - 1),
                                    )
                    for j in range(NJ):
                        for q in range(2):
                            col = half * TH + q * 512
                            _evac(
                                kt_sb[:, j, col : col + 512],
                                pk[j * 2 + q][:],
                                bk_sb,
                                j,
                                j * 2 + q,
                            )
                            _evac(
                                vt_sb[:, j, col : col + 512],
                                pv[j * 2 + q][:],
                                None,
                                j,
                                j * 2 + q + 1,
                            )

        def _transpose_v():
            # vt [m, t] -> vo [t, m] in [128,128] blocks on the PE.
            with tc.tile_pool(name="ptr", bufs=4, space="PSUM") as ptr:
                for tkc in range(NTK):
                    for j in range(NJ):
                        pt = ptr.tile([128, 128], FP16, tag="pt")
                        nc.tensor.transpose(
                            pt[:], vt_sb[:, j, ts(tkc, 128)], id_sb[:]
                        )
                        dst = vo_sb[:, tkc, :].rearrange(
                            "p (h x) -> p h x", x=VW
                        )[:, 2 * j : 2 * j + 2, 64 : 64 + DH]
                        nc.vector.tensor_copy(
                            dst, pt[:].rearrange("p (h d) -> p h d", d=DH)
                        )

        def _stage23():
            with (
                tc.tile_pool(name="sps", bufs=2, space="PSUM") as sps,
                tc.tile_pool(name="ops", bufs=2, space="PSUM") as ops,
                tc.tile_pool(name="epool", bufs=4) as epool,
                tc.tile_pool(name="xpool", bufs=8) as xpool,
                tc.tile_pool(name="npool", bufs=2) as npool,
            ):

                def _avs(h, otiles, tkc, pps):
                    for tqb in range(NTQB):
                        for q2 in range(TQB // 512):
                            nc.tensor.matmul(
                                otiles[tqb][:, ts(q2, 512)],
                                lhsT=vo_sb[:, tkc, h * VW : (h + 1) * VW],
                                rhs=pps[tqb][:, ts(q2, 512)],
                                start=(tkc == 0),
                                stop=(tkc == NTK - 1),
                            )

                for h in range(HPC):
                    j, hp = h // 2, 64 * (h % 2)
                    otiles = [
                        ops.tile([128, TQB], F32, tag="o", name=f"o{h}_{i}")
                        for i in range(NTQB)
                    ]
                    pending = []
                    et = None
                    for tkc in range(NTK):
                        if tkc % 2 == 0:
                            et = epool.tile([128, 2, T], FP16, tag="e")
                            nc.gpsimd.dma_start(
                                et[:],
                                d_E.ap()[h][
                                    tkc * 128 : (tkc + 2) * 128, :
                                ].rearrange("(a p) t -> p a t", a=2),
                            )
                        cur = []
                        for tqb in range(NTQB):
                            spt = sps.tile([128, TQB], F32, tag="s")
                            for q2 in range(TQB // 512):
                                nc.tensor.matmul(
                                    spt[:, ts(q2, 512)],
                                    lhsT=kt_sb[hp : hp + 64, j, ts(tkc, 128)],
                                    rhs=qt_sb[
                                        hp : hp + 64, j, tqb * TQB + q2 * 512 :
                                        tqb * TQB + (q2 + 1) * 512
                                    ],
                                    start=True,
                                    stop=True,
                                )
                            ex = xpool.tile([128, TQB], FP16, tag="ex")
                            nc.scalar.activation(ex[:], spt[:], Exp)
                            pp = xpool.tile([128, TQB], FP16, tag="pp")
                            nc.vector.tensor_mul(
                                pp[:], ex[:], et[:, tkc % 2, ts(tqb, TQB)]
                            )
                            cur.append(pp)
                        pending.append((tkc, cur))
                        if len(pending) > 3:
                            ptkc, pcur = pending.pop(0)
                            _avs(h, otiles, ptkc, pcur)
                    for ptkc, pcur in pending:
                        _avs(h, otiles, ptkc, pcur)

                    # denom lands on psum row 0 (ones column is first in
                    # the AV lhsT); dh rows start at partition 32. Everything
                    # quadrant-aligned: no partition shifts, no ACT involved.
                    for tqb in range(NTQB):
                        tq_sl = slice(tqb * TQB, (tqb + 1) * TQB)
                        rc = npool.tile([1, TQB], F32, tag="rc")
                        nc.vector.reciprocal_approx_fast(
                            rc[:], otiles[tqb][0:1, :]
                        )
                        rep = npool.tile([64, TQB], F32, tag="rep")
                        nc.gpsimd.partition_broadcast(rep[:], rc[:])
                        nc.vector.tensor_mul(
                            ot_sb[hp : hp + 64, j, tq_sl],
                            otiles[tqb][64 : 64 + DH, :],
                            rep[:],
                        )

        def _stage4():
            with (
                tc.tile_pool(name="pops", bufs=4, space="PSUM") as pops,
                tc.tile_pool(name="outst", bufs=2) as outst,
            ):
                for trp in range(T // 256):
                    ost = outst.tile([128, 2, DM], FP16, tag="ost")
                    for a in range(2):
                        tr = trp * 2 + a
                        for dmc in range(DM // 1024):
                            po = pops.tile([128, 1024], F32, tag="po")
                            for j in range(NJ):
                                for q2 in range(2):
                                    nc.tensor.matmul(
                                        po[:, ts(q2, 512)],
                                        lhsT=ot_sb[:, j, ts(tr, 128)],
                                        rhs=wo_sb[
                                            :, j,
                                            dmc * 1024 + q2 * 512 :
                                            dmc * 1024 + (q2 + 1) * 512,
                                        ],
                                        start=(j == 0),
                                        stop=(j == NJ - 1),
                                    )
                            if dmc % 2 == 0:
                                nc.scalar.activation(
                                    ost[:, a, ts(dmc, 1024)], po[:], Copy
                                )
                            else:
                                nc.vector.tensor_copy(
                                    ost[:, a, ts(dmc, 1024)], po[:]
                                )
                    nc.scalar.dma_start(
                        d_out.ap()[ts(trp, 256), :].rearrange(
                            "(a p) d -> p a d", a=2
                        ),
                        ost[:],
                    )

        def _compute_body():
            with tc.tile_pool(name="hin", bufs=3) as hin:
                _proj_kv_fused()
                _proj_pass(d_hqT, wq_sb, bq_sb, qt_sb, hin)
            _transpose_v()
            _stage23()
            _stage4()

        if reps > 1:
            hints = (
                mybir.EngineType.PE,
                mybir.EngineType.Activation,
                mybir.EngineType.DVE,
                mybir.EngineType.SP,
                mybir.EngineType.Pool,
            )
            with tc.For_i(0, reps, 1, hint_engines=hints):
                _compute_body()
        else:
            _compute_body()

    nc.compile()
    return nc


def _pack_w(wT, ndc):
    """[DM, m] (contraction-major) -> [128, ndc*m] SBUF-layout prepack."""
    dm, m = wT.shape
    return np.ascontiguousarray(
        wT.reshape(ndc, 128, m).transpose(1, 0, 2).reshape(128, ndc * m)
    )


def make_in_maps(hidden_q, hidden_kv, mask, position_bias, wq, bq, wk, bk, wv, wo):
    f16 = np.float16
    T = hidden_q.shape[1]
    DM = hidden_q.shape[2]
    NDC = DM // 128
    NJ = M // 128
    hqT = np.ascontiguousarray(hidden_q[0].T, dtype=f16)
    hkvT = np.ascontiguousarray(hidden_kv[0].T, dtype=f16)
    maskf = mask[0].astype(np.float32)
    in_maps = []
    for c in range(N_CORES):
        sl = slice(c * M, (c + 1) * M)
        wqT = (wq[sl] * (1.0 / np.sqrt(DH))).T.astype(f16)
        wkT = wk[sl].T.astype(f16)
        wvT = wv[sl].T.astype(f16)
        woT = wo[:, sl].T.astype(f16)
        pb_c = position_bias[0, c * HPC : (c + 1) * HPC]
        E = np.exp(pb_c, dtype=np.float32) * (maskf[None] * E_SCALE)
        E = np.ascontiguousarray(E.transpose(0, 2, 1)).astype(f16)
        in_maps.append(
            {
                "hqT": hqT,
                "hkvT": hkvT,
                "wqT": _pack_w(wqT, NDC),
                "wkT": _pack_w(wkT, NDC),
                "wvT": _pack_w(wvT, NDC),
                "woT": _pack_w(woT, NJ),
                "bq": np.ascontiguousarray(bq[sl] * (1.0 / np.sqrt(DH))),
                "bk": np.ascontiguousarray(bk[sl]),
                "E": E,
            }
        )
    return in_maps


def kernel(hidden_q, hidden_kv, mask, position_bias, wq, bq, wk, bk, wv, bv, wo, bo):
    hidden_q = np.asarray(hidden_q, np.float32)
    hidden_kv = np.asarray(hidden_kv, np.float32)
    mask = np.asarray(mask)
    position_bias = np.asarray(position_bias, np.float32)
    wq, bq = np.asarray(wq, np.float32), np.asarray(bq, np.float32)
    wk, bk = np.asarray(wk, np.float32), np.asarray(bk, np.float32)
    wv, bv = np.asarray(wv, np.float32), np.asarray(bv, np.float32)
    wo, bo = np.asarray(wo, np.float32), np.asarray(bo, np.float32)

    T = hidden_q.shape[1]
    DM = hidden_q.shape[2]

    with_bias = bool(np.any(bq) or np.any(bk))
    nc = build_attention_nc(T, DM, with_bias=with_bias)
    in_maps = make_in_maps(
        hidden_q, hidden_kv, mask, position_bias, wq, bq, wk, bk, wv, wo
    )
    res = run_bass_kernel_spmd(nc, in_maps, core_ids=list(range(N_CORES)))
    if res.exec_time_ns is not None:
        print(f"HW exec time: {res.exec_time_ns} ns")

    out = np.zeros((T, DM), np.float64)
    for c in range(N_CORES):
        out += res.results[c]["out"].astype(np.float64)
    out += (bv.astype(np.float64) @ wo.T.astype(np.float64)) + bo.astype(np.float64)
    return out[None].astype(np.float32)


# revision 8
# speedup vs baseline: 4.6558x; 1.3447x over previous
"""Trainium2 Bass kernel for multi-head attention (nn_Attention_39573828665669).

Reference computation (per batch b=1, SQ=SKV=2048, DM=2048, H=32, DH=64):
    q = hidden_q @ wq.T + bq ; k = hidden_kv @ wk.T + bk ; v = hidden_kv @ wv.T + bv
    score = q @ k.T / sqrt(DH) + position_bias ; masked softmax ; out = probs @ v
    return out @ wo.T + bo

Sharding: tensor-parallel over heads. Each of the 8 cores handles 4 heads
(256 of the 2048 head dims), computes its partial output projection
out_c = attn_c @ wo[:, cols_c].T, and the host sums the 8 partials.

Device-side formulation (per core), all matmuls fp16 with fp32 psum:
  - 1/sqrt(DH) folded into wq on the host; mask and position_bias folded
    into E = exp(pb) * mask / 256 (fp16, [tk, tq] per head). The device
    computes P' = exp(S) * E unnormalized; a leading ones-column in the
    V block makes the AV matmul emit the softmax denominator on psum
    partition 0 (per-head AV block layout: [1 ones | 63 zero | 64 dh],
    which keeps every later access quadrant-aligned).
  - 1/denom via DVE reciprocal_approx_fast at partition 0 + Pool
    partition_broadcast; normalization fuses into the psum evacuation
    multiply on DVE. No ACT table swaps anywhere (exp/copy share a table).
  - bv is folded on the host as out += bv @ wo.T; bo added on the host.

Schedule, tuned against the TimelineSim cost model:
  - K and V^T share one streaming pass over hkvT (half-T psum split:
    4 banks K + 4 banks V^T); V^T is then transposed to [t, m] with 32
    [128,128] PE transposes through fp16 psum. Q uses a full-T pass.
    All projection passes are 512-wide-rhs, d-chunk-outer, full-psum.
  - DMA is batched 2 chunks per trigger (8 KB per partition line) and
    spread across queues: hq/hkv on SP, weights+E on Pool, out on ACT.
    E prefetches during stage 1; hq prefetches during the KV pass.
  - Stage 2/3 per head: S (2x512-wide per tkc/tqb) -> ACT exp ->
    DVE mul by E -> AV, with AV lagging three tkc so the PE never waits
    on the exp/mul chain. ACT runs ~100% busy here and is the pacer.
  - Output projection accumulates [128,1024] psum tiles (2 banks, 4
    matmuls) with ACT/DVE-alternating evacuation and [128,2,DM] stores.
"""

import os
from contextlib import ExitStack

import numpy as np

import concourse.bass as bass
import concourse.tile as tile
from concourse import bacc, mybir
from concourse.bass_utils import run_bass_kernel_spmd
from concourse.masks import make_identity

F32 = mybir.dt.float32
FP16 = mybir.dt.float16

ts = bass.ts

N_CORES = 8
H = 32
DH = 64
HPC = H // N_CORES          # heads per core = 4
M = HPC * DH                # per-core head dims = 256
E_SCALE = 1.0 / 256.0


def build_attention_nc(T: int, DM: int, reps: int = 1, with_bias: bool = False):
    assert HPC == 4 and DH == 64
    NJ = M // 128            # m chunks (2)
    NDC = DM // 128          # contraction chunks for projections
    NTK = T // 128           # key-position chunks
    NTQ = T // 512           # 512-wide query chunks
    TQB = min(1024, T)
    NTQB = T // TQB

    nc = bacc.Bacc("TRN2", target_bir_lowering=False, debug=False)

    d_hqT = nc.declare_dram_parameter("hqT", [DM, T], FP16, isOutput=False)
    d_hkvT = nc.declare_dram_parameter("hkvT", [DM, T], FP16, isOutput=False)
    d_wq = nc.declare_dram_parameter("wqT", [128, NDC * M], FP16, isOutput=False)
    d_wk = nc.declare_dram_parameter("wkT", [128, NDC * M], FP16, isOutput=False)
    d_wv = nc.declare_dram_parameter("wvT", [128, NDC * M], FP16, isOutput=False)
    d_wo = nc.declare_dram_parameter("woT", [128, NJ * DM], FP16, isOutput=False)
    d_bq = nc.declare_dram_parameter("bq", [M], F32, isOutput=False)
    d_bk = nc.declare_dram_parameter("bk", [M], F32, isOutput=False)
    d_E = nc.declare_dram_parameter("E", [HPC, T, T], FP16, isOutput=False)
    d_out = nc.declare_dram_parameter("out", [T, DM], FP16, isOutput=True)

    Exp = mybir.ActivationFunctionType.Exp
    Ln = mybir.ActivationFunctionType.Ln
    Copy = mybir.ActivationFunctionType.Copy
    Identity = mybir.ActivationFunctionType.Identity

    with tile.TileContext(nc) as tc, ExitStack() as ctx:
        wpool = ctx.enter_context(tc.tile_pool(name="weights", bufs=1))
        spool = ctx.enter_context(tc.tile_pool(name="state", bufs=1))

        wq_sb = wpool.tile([128, NDC, M], FP16, tag="wq")
        wk_sb = wpool.tile([128, NDC, M], FP16, tag="wk")
        wv_sb = wpool.tile([128, NDC, M], FP16, tag="wv")
        wo_sb = wpool.tile([128, NJ, DM], FP16, tag="wo")
        bq_sb = wpool.tile([128, NJ], F32, tag="bq")
        bk_sb = wpool.tile([128, NJ], F32, tag="bk")
        id_sb = wpool.tile([128, 128], FP16, tag="idn")
        # weights on the Pool queue so they overlap the first hq tiles (SP)
        nc.gpsimd.dma_start(
            wk_sb[:, 0:1, :].rearrange("p a b -> p (a b)"), d_wk.ap()[:, 0:M]
        )
        nc.gpsimd.dma_start(
            wv_sb[:, 0:1, :].rearrange("p a b -> p (a b)"), d_wv.ap()[:, 0:M]
        )
        nc.gpsimd.dma_start(
            wk_sb[:, 1:, :].rearrange("p a b -> p (a b)"), d_wk.ap()[:, M:]
        )
        nc.gpsimd.dma_start(
            wv_sb[:, 1:, :].rearrange("p a b -> p (a b)"), d_wv.ap()[:, M:]
        )
        nc.gpsimd.dma_start(wq_sb[:].rearrange("p a b -> p (a b)"), d_wq.ap())
        nc.gpsimd.dma_start(wo_sb[:].rearrange("p a b -> p (a b)"), d_wo.ap())
        nc.sync.dma_start(bq_sb[:], d_bq.ap().rearrange("(j p) -> p j", p=128))
        nc.sync.dma_start(bk_sb[:], d_bk.ap().rearrange("(j p) -> p j", p=128))
        make_identity(nc, id_sb[:])

        qt_sb = spool.tile([128, NJ, T], FP16, tag="qt")     # Q.T / sqrt(DH)
        kt_sb = spool.tile([128, NJ, T], FP16, tag="kt")     # K.T
        vt_sb = spool.tile([128, NJ, T], FP16, tag="vt")     # V.T (pre-transpose)
        VW = 128  # per-head AV lhsT width: [1 ones | 63 pad | 64 dh]
        vo_sb = spool.tile([128, NTK, HPC * VW], FP16, tag="vo")
        ot_sb = spool.tile([128, NJ, T], FP16, tag="ot")     # normalized O.T

        nc.vector.memset(vo_sb[:], 0.0)
        ones_view = vo_sb[:].rearrange("p n (h x) -> p n h x", x=VW)[:, :, :, 0:1]
        nc.vector.memset(ones_view, 1.0)

        def _evac(dst, src, b_sb, j, idx):
            if with_bias and b_sb is not None:
                nc.scalar.activation(dst, src, Identity, bias=b_sb[:, j : j + 1])
            elif idx % 2 == 0:
                nc.scalar.activation(dst, src, Copy)
            else:
                nc.vector.tensor_copy(dst, src)

        def _proj_pass(d_src, w_sb, b_sb, dst_sb, hin):
            # d-chunk-outer projection: full-T psum (NJ * NTQ = 8 banks),
            # one [128, 2, T] rhs DMA per dc pair (8 KB per partition line).
            with tc.tile_pool(name="pqk", bufs=8, space="PSUM") as pqk:
                pp = [
                    pqk.tile([128, 512], F32, tag="pp", name=f"pp{i}")
                    for i in range(NJ * NTQ)
                ]
                for dcp in range(NDC // 2):
                    ht = hin.tile([128, 2, T], FP16, tag="h")
                    nc.sync.dma_start(
                        ht[:],
                        d_src.ap()[ts(dcp, 256), :].rearrange(
                            "(a p) t -> p a t", a=2
                        ),
                    )
                    for a in range(2):
                        dc = dcp * 2 + a
                        for j in range(NJ):
                            for q in range(NTQ):
                                nc.tensor.matmul(
                                    pp[j * NTQ + q][:],
                                    lhsT=w_sb[:, dc, ts(j, 128)],
                                    rhs=ht[:, a, ts(q, 512)],
                                    start=(dc == 0),
                                    stop=(dc == NDC - 1),
                                )
                for j in range(NJ):
                    for q in range(NTQ):
                        _evac(
                            dst_sb[:, j, ts(q, 512)],
                            pp[j * NTQ + q][:],
                            b_sb,
                            j,
                            j * NTQ + q,
                        )

        def _proj_kv_fused():
            # K and V^T share one streaming pass over hkvT: half-T psum per
            # projection (4 banks K + 4 banks V^T), two column-halves.
            TH = T // 2
            for half in range(2):
                with (
                    tc.tile_pool(name="pkv", bufs=4, space="PSUM") as pkv,
                    tc.tile_pool(name="hin2", bufs=3) as hin2,
                ):
                    pk = [
                        pkv.tile([128, 512], F32, tag="pk", name=f"pk{half}_{i}")
                        for i in range(NJ * 2)
                    ]
                    pv = [
                        pkv.tile([128, 512], F32, tag="pv", name=f"pv{half}_{i}")
                        for i in range(NJ * 2)
                    ]
                    for dcp in range(NDC // 2):
                        ht = hin2.tile([128, 2, TH], FP16, tag="h2")
                        nc.sync.dma_start(
                            ht[:],
                            d_hkvT.ap()[
                                ts(dcp, 256), half * TH : (half + 1) * TH
                            ].rearrange("(a p) t -> p a t", a=2),
                        )
                        for a in range(2):
                            dc = dcp * 2 + a
                            for j in range(NJ):
                                for q in range(2):
                                    nc.tensor.matmul(
                                        pk[j * 2 + q][:],
                                        lhsT=wk_sb[:, dc, ts(j, 128)],
                                        rhs=ht[:, a, ts(q, 512)],
                                        start=(dc == 0),
                                        stop=(dc == NDC - 1),
                                    )
                                    nc.tensor.matmul(
                                        pv[j * 2 + q][:],
                                        lhsT=wv_sb[:, dc, ts(j, 128)],
                                        rhs=ht[:, a, ts(q, 512)],
                                        start=(dc == 0),
                                        stop=(dc == NDC - 1),
                                    )
                    for j in range(NJ):
                        for q in range(2):
                            col = half * TH + q * 512
                            _evac(
                                kt_sb[:, j, col : col + 512],
                                pk[j * 2 + q][:],
                                bk_sb,
                                j,
                                j * 2 + q,
                            )
                            _evac(
                                vt_sb[:, j, col : col + 512],
                                pv[j * 2 + q][:],
                                None,
                                j,
                                j * 2 + q + 1,
                            )

        def _transpose_v():
            # vt [m, t] -> vo [t, m] in [128,128] blocks on the PE.
            with tc.tile_pool(name="ptr", bufs=4, space="PSUM") as ptr:
                for tkc in range(NTK):
                    for j in range(NJ):
                        pt = ptr.tile([128, 128], FP16, tag="pt")
                        nc.tensor.transpose(
                            pt[:], vt_sb[:, j, ts(tkc, 128)], id_sb[:]
                        )
                        dst = vo_sb[:, tkc, :].rearrange(
                            "p (h x) -> p h x", x=VW
                        )[:, 2 * j : 2 * j + 2, 64 : 64 + DH]
                        nc.vector.tensor_copy(
                            dst, pt[:].rearrange("p (h d) -> p h d", d=DH)
                        )

        def _stage23():
            with (
                tc.tile_pool(name="sps", bufs=2, space="PSUM") as sps,
                tc.tile_pool(name="ops", bufs=2, space="PSUM") as ops,
                tc.tile_pool(name="epool", bufs=4) as epool,
                tc.tile_pool(name="xpool", bufs=8) as xpool,
                tc.tile_pool(name="npool", bufs=2) as npool,
            ):

                def _avs(h, otiles, tkc, pps):
                    for tqb in range(NTQB):
                        for q2 in range(TQB // 512):
                            nc.tensor.matmul(
                                otiles[tqb][:, ts(q2, 512)],
                                lhsT=vo_sb[:, tkc, h * VW : (h + 1) * VW],
                                rhs=pps[tqb][:, ts(q2, 512)],
                                start=(tkc == 0),
                                stop=(tkc == NTK - 1),
                            )

                for h in range(HPC):
                    j, hp = h // 2, 64 * (h % 2)
                    otiles = [
                        ops.tile([128, TQB], F32, tag="o", name=f"o{h}_{i}")
                        for i in range(NTQB)
                    ]
                    pending = []
                    et = None
                    for tkc in range(NTK):
                        if tkc % 2 == 0:
                            et = epool.tile([128, 2, T], FP16, tag="e")
                            nc.gpsimd.dma_start(
                                et[:],
                                d_E.ap()[h][
                                    tkc * 128 : (tkc + 2) * 128, :
                                ].rearrange("(a p) t -> p a t", a=2),
                            )
                        cur = []
                        for tqb in range(NTQB):
                            spt = sps.tile([128, TQB], F32, tag="s")
                            for q2 in range(TQB // 512):
                                nc.tensor.matmul(
                                    spt[:, ts(q2, 512)],
                                    lhsT=kt_sb[hp : hp + 64, j, ts(tkc, 128)],
                                    rhs=qt_sb[
                                        hp : hp + 64, j, tqb * TQB + q2 * 512 :
                                        tqb * TQB + (q2 + 1) * 512
                                    ],
                                    start=True,
                                    stop=True,
                                )
                            ex = xpool.tile([128, TQB], FP16, tag="ex")
                            nc.scalar.activation(ex[:], spt[:], Exp)
                            pp = xpool.tile([128, TQB], FP16, tag="pp")
                            nc.vector.tensor_mul(
                                pp[:], ex[:], et[:, tkc % 2, ts(tqb, TQB)]
                            )
                            cur.append(pp)
                        pending.append((tkc, cur))
                        if len(pending) > 3:
                            ptkc, pcur = pending.pop(0)
                            _avs(h, otiles, ptkc, pcur)
                    for ptkc, pcur in pending:
                        _avs(h, otiles, ptkc, pcur)

                    # denom lands on psum row 0 (ones column is first in
                    # the AV lhsT); dh rows start at partition 32. Everything
                    # quadrant-aligned: no partition shifts, no ACT involved.
                    for tqb in range(NTQB):
                        tq_sl = slice(tqb * TQB, (tqb + 1) * TQB)
                        rc = npool.tile([1, TQB], F32, tag="rc")
                        nc.vector.reciprocal_approx_fast(
                            rc[:], otiles[tqb][0:1, :]
                        )
                        rep = npool.tile([64, TQB], F32, tag="rep")
                        nc.gpsimd.partition_broadcast(rep[:], rc[:])
                        nc.vector.tensor_mul(
                            ot_sb[hp : hp + 64, j, tq_sl],
                            otiles[tqb][64 : 64 + DH, :],
                            rep[:],
                        )

        def _stage4():
            with (
                tc.tile_pool(name="pops", bufs=4, space="PSUM") as pops,
                tc.tile_pool(name="outst", bufs=2) as outst,
            ):
                for trp in range(T // 256):
                    ost = outst.tile([128, 2, DM], FP16, tag="ost")
                    for a in range(2):
                        tr = trp * 2 + a
                        for dmc in range(DM // 1024):
                            po = pops.tile([128, 1024], F32, tag="po")
                            for j in range(NJ):
                                for q2 in range(2):
                                    nc.tensor.matmul(
                                        po[:, ts(q2, 512)],
                                        lhsT=ot_sb[:, j, ts(tr, 128)],
                                        rhs=wo_sb[
                                            :, j,
                                            dmc * 1024 + q2 * 512 :
                                            dmc * 1024 + (q2 + 1) * 512,
                                        ],
                                        start=(j == 0),
                                        stop=(j == NJ - 1),
                                    )
                            if dmc % 2 == 0:
                                nc.scalar.activation(
                                    ost[:, a, ts(dmc, 1024)], po[:], Copy
                                )
                            else:
                                nc.vector.tensor_copy(
                                    ost[:, a, ts(dmc, 1024)], po[:]
                                )
                    nc.scalar.dma_start(
                        d_out.ap()[ts(trp, 256), :].rearrange(
                            "(a p) d -> p a d", a=2
                        ),
                        ost[:],
                    )

        def _compute_body():
            with tc.tile_pool(name="hin", bufs=3) as hin:
                _proj_kv_fused()
                _proj_pass(d_hqT, wq_sb, bq_sb, qt_sb, hin)
            _transpose_v()
            _stage23()
            _stage4()

        if reps > 1:
            hints = (
                mybir.EngineType.PE,
                mybir.EngineType.Activation,
                mybir.EngineType.DVE,
                mybir.EngineType.SP,
                mybir.EngineType.Pool,
            )
            with tc.For_i(0, reps, 1, hint_engines=hints):
                _compute_body()
        else:
            _compute_body()

    nc.compile()
    return nc


def _pack_w(wT, ndc):
    """[DM, m] (contraction-major) -> [128, ndc*m] SBUF-layout prepack."""
    dm, m = wT.shape
    return np.ascontiguousarray(
        wT.reshape(ndc, 128, m).transpose(1, 0, 2).reshape(128, ndc * m)
    )


def make_in_maps(hidden_q, hidden_kv, mask, position_bias, wq, bq, wk, bk, wv, wo):
    f16 = np.float16
    T = hidden_q.shape[1]
    DM = hidden_q.shape[2]
    NDC = DM // 128
    NJ = M // 128
    hqT = np.ascontiguousarray(hidden_q[0].T, dtype=f16)
    hkvT = np.ascontiguousarray(hidden_kv[0].T, dtype=f16)
    maskf = mask[0].astype(np.float32)
    in_maps = []
    for c in range(N_CORES):
        sl = slice(c * M, (c + 1) * M)
        wqT = (wq[sl] * (1.0 / np.sqrt(DH))).T.astype(f16)
        wkT = wk[sl].T.astype(f16)
        wvT = wv[sl].T.astype(f16)
        woT = wo[:, sl].T.astype(f16)
        pb_c = position_bias[0, c * HPC : (c + 1) * HPC]
        E = np.exp(pb_c, dtype=np.float32) * (maskf[None] * E_SCALE)
        E = np.ascontiguousarray(E.transpose(0, 2, 1)).astype(f16)
        in_maps.append(
            {
                "hqT": hqT,
                "hkvT": hkvT,
                "wqT": _pack_w(wqT, NDC),
                "wkT": _pack_w(wkT, NDC),
                "wvT": _pack_w(wvT, NDC),
                "woT": _pack_w(woT, NJ),
                "bq": np.ascontiguousarray(bq[sl] * (1.0 / np.sqrt(DH))),
                "bk": np.ascontiguousarray(bk[sl]),
                "E": E,
            }
        )
    return in_maps


def kernel(hidden_q, hidden_kv, mask, position_bias, wq, bq, wk, bk, wv, bv, wo, bo):
    hidden_q = np.asarray(hidden_q, np.float32)
    hidden_kv = np.asarray(hidden_kv, np.float32)
    mask = np.asarray(mask)
    position_bias = np.asarray(position_bias, np.float32)
    wq, bq = np.asarray(wq, np.float32), np.asarray(bq, np.float32)
    wk, bk = np.asarray(wk, np.float32), np.asarray(bk, np.float32)
    wv, bv = np.asarray(wv, np.float32), np.asarray(bv, np.float32)
    wo, bo = np.asarray(wo, np.float32), np.asarray(bo, np.float32)

    T = hidden_q.shape[1]
    DM = hidden_q.shape[2]

    with_bias = bool(np.any(bq) or np.any(bk))
    nc = build_attention_nc(T, DM, with_bias=with_bias)
    in_maps = make_in_maps(
        hidden_q, hidden_kv, mask, position_bias, wq, bq, wk, bk, wv, wo
    )
    res = run_bass_kernel_spmd(nc, in_maps, core_ids=list(range(N_CORES)))
    if res.exec_time_ns is not None:
        print(f"HW exec time: {res.exec_time_ns} ns")

    out = np.zeros((T, DM), np.float64)
    for c in range(N_CORES):
        out += res.results[c]["out"].astype(np.float64)
    out += (bv.astype(np.float64) @ wo.T.astype(np.float64)) + bo.astype(np.float64)
    return out[None].astype(np.float32)
